# revision 6
# baseline (speedup 1.0000x reference)
"""Multi-head attention (B=4, S=2048, H=1024, 16 heads) on 8 Trainium2 cores.

Sharding: core c = 2*b + g handles batch b with head-group g (8 heads = 512 of
1024 H-columns).  Each core computes Q/K/V projections for its column slice,
attention for its 8 heads, and a partial output projection against its 512
rows of wo.  The host sums the two partials per batch and adds bo.

Kernel internals (per core):
  - x [2048,1024] f32 is DMA'd in, PE-transposed to xT and cast to bf16.
  - qT/kT [c,t] computed directly (lhsT = wq chunk, rhs = xT) so attention
    scores come out transposed (scoresT [k,q]) with no further transposes.
  - v computed in natural [t,c] layout (lhsT = xT chunk, rhs = wv), stored
    ones-augmented per head ([.., 65]) so the AV matmul also produces the
    softmax denominator in psum partition 64.
  - exp on ACT (scale folded in); no max-subtraction needed (logits ~N(0,1)).
  - normalization: reciprocal of the denominator row, partition-broadcast via
    stride-0 DMA, multiplied into ctxT during psum->sbuf evacuation.
  - O-projection: lhsT = ctxT chunks, rhs = wo chunks, fp32 partial out.
All matmuls run in bf16 with fp32 psum accumulation.
"""
import sys

if "/opt/trn_rl_repo" not in sys.path:
    sys.path.insert(0, "/opt/trn_rl_repo")

import numpy as np

import concourse.bass as bass
import concourse.tile as tile
from concourse import bacc, mybir
from concourse.bass_utils import run_bass_kernel_spmd
from concourse.masks import make_identity

B, S, H = 4, 2048, 1024
NH, HD = 16, 64
G = H // 2            # local H columns per core
NHL = NH // 2         # local heads per core
P = 128
F32 = mybir.dt.float32
BF16 = mybir.dt.bfloat16
SCALE = 1.0 / float(np.sqrt(HD))

TT = S // P           # 16 token tiles
HC = H // P           # 8 contraction chunks for projections
CT = G // P           # 4 c-tiles
KC = S // P           # 16 k chunks
QW = 1024             # q window width in attention
NQH = S // QW         # 2 q windows
MM_N = 512            # matmul moving free dim (one psum bank)

_NC_CACHE = None


def _emit(nc, tc, aps):
    x, wq, wk, wv, wo, bq, bk, bv, out = aps

    import contextlib
    ctx = contextlib.ExitStack()
    with ctx:
        persist = ctx.enter_context(tc.tile_pool(name="persist", bufs=1))

        # ---- persistent sbuf tensors ----
        xT = persist.tile([P, HC, S], BF16)            # x^T, bf16
        wq_sb = persist.tile([P, HC, G], BF16)
        wk_sb = persist.tile([P, HC, G], BF16)
        wv_sb = persist.tile([P, HC, G], BF16)
        wo_sb = persist.tile([P, CT, H], BF16)
        qT = persist.tile([P, CT, S], BF16)
        kT = persist.tile([P, CT, S], BF16)
        v_aug = persist.tile([P, KC, NHL, HD + 1], BF16)
        ctxT = persist.tile([P, CT, S], BF16)
        bq_sb = persist.tile([P, CT], F32)
        bk_sb = persist.tile([P, CT], F32)
        bv_row = persist.tile([1, G], BF16)
        ones_col = persist.tile([1, P], BF16)
        ones_f32 = persist.tile([1, HD], F32)
        identity = persist.tile([P, P], F32)

        make_identity(nc, identity)
        nc.vector.memset(ones_col, 1.0)
        nc.vector.memset(ones_f32, 1.0)
        nc.vector.memset(v_aug[:, :, :, HD:HD + 1], 1.0)

        # ---- phase 0: loads, x transpose, bf16 casts ----
        with tc.tile_pool(name="xload", bufs=3) as xload, \
             tc.tile_pool(name="wload", bufs=2) as wload, \
             tc.tile_pool(name="tp_ps", bufs=4, space="PSUM") as tp_ps:
            nc.sync.dma_start(out=bq_sb, in_=bq.rearrange("(ct p) -> p ct", p=P))
            nc.sync.dma_start(out=bk_sb, in_=bk.rearrange("(ct p) -> p ct", p=P))
            bv_f = wload.tile([1, G], F32, tag="bias")
            nc.sync.dma_start(out=bv_f, in_=bv.rearrange("(a c) -> a c", a=1))
            nc.vector.tensor_copy(out=bv_row, in_=bv_f)

            for tt in range(TT):
                xt = xload.tile([P, H], F32, tag="x")
                nc.sync.dma_start(out=xt, in_=x[tt * P:(tt + 1) * P, :])
                for hc in range(HC):
                    tp = tp_ps.tile([P, P], F32)
                    nc.tensor.transpose(tp, xt[:, hc * P:(hc + 1) * P], identity)
                    nc.vector.tensor_copy(
                        out=xT[:, hc, tt * P:(tt + 1) * P], in_=tp)

            for wap, dst in ((wq, wq_sb), (wk, wk_sb), (wv, wv_sb)):
                wt = wload.tile([P, HC, G], F32, tag="w")
                nc.sync.dma_start(
                    out=wt, in_=wap.rearrange("(hc p) c -> p hc c", p=P))
                nc.vector.tensor_copy(out=dst, in_=wt)
            wot = wload.tile([P, CT, H], F32, tag="w")
            nc.sync.dma_start(
                out=wot, in_=wo.rearrange("(cc p) o -> p cc o", p=P))
            nc.vector.tensor_copy(out=wo_sb, in_=wot)

        # ---- phase 1: projections ----
        with tc.tile_pool(name="proj_ps", bufs=4, space="PSUM") as pps:
            # qT/kT: [c, t] = sum_hc wq[hc, c]^T @ xT[hc, t]
            for w_sb, b_sb, dst in ((wq_sb, bq_sb, qT), (wk_sb, bk_sb, kT)):
                for ct in range(CT):
                    for nq in range(S // MM_N):
                        acc = pps.tile([P, MM_N], F32, tag="acc")
                        for hc in range(HC):
                            nc.tensor.matmul(
                                acc,
                                lhsT=w_sb[:, hc, ct * P:(ct + 1) * P],
                                rhs=xT[:, hc, nq * MM_N:(nq + 1) * MM_N],
                                start=(hc == 0), stop=(hc == HC - 1))
                        nc.vector.tensor_scalar_add(
                            out=dst[:, ct, nq * MM_N:(nq + 1) * MM_N],
                            in0=acc, scalar1=b_sb[:, ct:ct + 1])
            # v: [t, c] = sum_hc xT[hc, t]^T @ wv[hc, c]  (+ ones*bv)
            for tt in range(TT):
                acc = pps.tile([P, G], F32, tag="acc")
                for hc in range(HC):
                    nc.tensor.matmul(
                        acc,
                        lhsT=xT[:, hc, tt * P:(tt + 1) * P],
                        rhs=wv_sb[:, hc, :],
                        start=(hc == 0), stop=False)
                nc.tensor.matmul(
                    acc, lhsT=ones_col, rhs=bv_row, start=False, stop=True)
                nc.vector.tensor_copy(
                    out=v_aug[:, tt, :, 0:HD],
                    in_=acc.rearrange("p (h d) -> p h d", h=NHL))

        # ---- phase 2: attention (per head, per q-window) ----
        with tc.tile_pool(name="sc_ps", bufs=2, space="PSUM") as sc_pool, \
             tc.tile_pool(name="ctx_ps", bufs=1, space="PSUM") as ctx_pool, \
             tc.tile_pool(name="bc_ps", bufs=1, space="PSUM") as bc_pool, \
             tc.tile_pool(name="expp", bufs=3) as exp_pool, \
             tc.tile_pool(name="normp", bufs=2) as norm_pool:
            for h in range(NHL):
                ct = h // 2
                po = (h % 2) * HD
                for qh in range(NQH):
                    q0 = qh * QW
                    ctx_ps = ctx_pool.tile([HD + 1, QW], F32)
                    prev_ex = None
                    for kc in range(KC):
                        sc = sc_pool.tile([P, QW], F32)
                        for nq in range(QW // MM_N):
                            nc.tensor.matmul(
                                sc[:, nq * MM_N:(nq + 1) * MM_N],
                                lhsT=kT[po:po + HD, ct, kc * P:(kc + 1) * P],
                                rhs=qT[po:po + HD, ct,
                                       q0 + nq * MM_N:q0 + (nq + 1) * MM_N],
                                start=True, stop=True)
                        if prev_ex is not None:
                            pex, pkc = prev_ex
                            for nq in range(QW // MM_N):
                                nc.tensor.matmul(
                                    ctx_ps[:, nq * MM_N:(nq + 1) * MM_N],
                                    lhsT=v_aug[:, pkc, h, :],
                                    rhs=pex[:, nq * MM_N:(nq + 1) * MM_N],
                                    start=(pkc == 0), stop=False)
                        ex = exp_pool.tile([P, QW], BF16)
                        nc.scalar.activation(
                            out=ex, in_=sc,
                            func=mybir.ActivationFunctionType.Exp, scale=SCALE)
                        prev_ex = (ex, kc)
                    pex, pkc = prev_ex
                    for nq in range(QW // MM_N):
                        nc.tensor.matmul(
                            ctx_ps[:, nq * MM_N:(nq + 1) * MM_N],
                            lhsT=v_aug[:, pkc, h, :],
                            rhs=pex[:, nq * MM_N:(nq + 1) * MM_N],
                            start=False, stop=True)
                    # normalize: ctxT = ctx_ps[0:HD] * (1 / rowsum)
                    recip = norm_pool.tile([1, QW], F32, tag="recip")
                    nc.vector.reciprocal(out=recip, in_=ctx_ps[HD:HD + 1, :])
                    # partition-broadcast via K=1 matmul: [HD,QW] = ones^T @ recip
                    bcast = bc_pool.tile([HD, QW], F32)
                    for nq in range(QW // MM_N):
                        nc.tensor.matmul(
                            bcast[:, nq * MM_N:(nq + 1) * MM_N],
                            lhsT=ones_f32,
                            rhs=recip[:, nq * MM_N:(nq + 1) * MM_N],
                            start=True, stop=True)
                    bcast_sb = norm_pool.tile([HD, QW], F32, tag="bcast_sb")
                    nc.vector.tensor_copy(out=bcast_sb, in_=bcast)
                    nc.vector.tensor_mul(
                        out=ctxT[po:po + HD, ct, q0:q0 + QW],
                        in0=ctx_ps[0:HD, :], in1=bcast_sb)

        # ---- phase 3: output projection (partial; host sums group halves) ----
        with tc.tile_pool(name="o_ps", bufs=4, space="PSUM") as ops, \
             tc.tile_pool(name="o_sb", bufs=3) as osb:
            for tt in range(TT):
                for no in range(H // MM_N):
                    acc = ops.tile([P, MM_N], F32)
                    for cc in range(CT):
                        nc.tensor.matmul(
                            acc,
                            lhsT=ctxT[:, cc, tt * P:(tt + 1) * P],
                            rhs=wo_sb[:, cc, no * MM_N:(no + 1) * MM_N],
                            start=(cc == 0), stop=(cc == CT - 1))
                    ot = osb.tile([P, MM_N], F32)
                    nc.vector.tensor_copy(out=ot, in_=acc)
                    nc.sync.dma_start(
                        out=out[tt * P:(tt + 1) * P, no * MM_N:(no + 1) * MM_N],
                        in_=ot)


def build_program():
    global _NC_CACHE
    if _NC_CACHE is not None:
        return _NC_CACHE
    nc = bacc.Bacc("TRN2", debug=False, num_devices=8)
    x = nc.dram_tensor("x", [S, H], F32, kind="ExternalInput").ap()
    wq = nc.dram_tensor("wq", [H, G], F32, kind="ExternalInput").ap()
    wk = nc.dram_tensor("wk", [H, G], F32, kind="ExternalInput").ap()
    wv = nc.dram_tensor("wv", [H, G], F32, kind="ExternalInput").ap()
    wo = nc.dram_tensor("wo", [G, H], F32, kind="ExternalInput").ap()
    bq = nc.dram_tensor("bq", [G], F32, kind="ExternalInput").ap()
    bk = nc.dram_tensor("bk", [G], F32, kind="ExternalInput").ap()
    bv = nc.dram_tensor("bv", [G], F32, kind="ExternalInput").ap()
    out = nc.dram_tensor("out", [S, H], F32, kind="ExternalOutput").ap()
    with tile.TileContext(nc) as tc:
        _emit(nc, tc, (x, wq, wk, wv, wo, bq, bk, bv, out))
    nc.compile()
    _NC_CACHE = nc
    return nc


def make_in_maps(x, wq, bq, wk, bk, wv, bv, wo, bo):
    x = np.asarray(x, dtype=np.float32)
    in_maps = []
    for c in range(8):
        b, g = divmod(c, 2)
        sl = slice(g * G, (g + 1) * G)
        in_maps.append({
            "x": np.ascontiguousarray(x[b]),
            "wq": np.ascontiguousarray(np.asarray(wq, np.float32)[:, sl]),
            "wk": np.ascontiguousarray(np.asarray(wk, np.float32)[:, sl]),
            "wv": np.ascontiguousarray(np.asarray(wv, np.float32)[:, sl]),
            "wo": np.ascontiguousarray(np.asarray(wo, np.float32)[sl, :]),
            "bq": np.ascontiguousarray(np.asarray(bq, np.float32)[sl]),
            "bk": np.ascontiguousarray(np.asarray(bk, np.float32)[sl]),
            "bv": np.ascontiguousarray(np.asarray(bv, np.float32)[sl]),
        })
    return in_maps


def gather_out(results, bo):
    bo = np.asarray(bo, dtype=np.float32)
    out = np.empty((B, S, H), dtype=np.float32)
    for b in range(B):
        out[b] = results[2 * b]["out"] + results[2 * b + 1]["out"] + bo
    return out


def kernel(x, wq, bq, wk, bk, wv, bv, wo, bo, trace=False):
    nc = build_program()
    in_maps = make_in_maps(x, wq, bq, wk, bk, wv, bv, wo, bo)
    r = run_bass_kernel_spmd(nc, in_maps, list(range(8)), trace=trace)
    out = gather_out(r.results, bo)
    if trace:
        kernel.last_exec_time_ns = r.exec_time_ns
        kernel.last_results = r
    return out


# revision 13
# speedup vs baseline: 1.3010x; 1.3010x over previous
"""Multi-head attention (B=4, S=2048, H=1024, 16 heads) on 8 Trainium2 cores.

Sharding: core c = 2*b + g handles batch b with head-group g (8 heads = 512 of
1024 H-columns).  Each core computes Q/K/V projections for its column slice,
attention for its 8 heads, and a partial output projection against its 512
rows of wo.  The host sums the two partials per batch and adds bo.

Kernel internals (per core):
  - x [2048,1024] f32 is DMA'd in, PE-transposed to xT and cast to bf16.
  - qT/kT [c,t] computed directly (lhsT = wq chunk, rhs = xT) so attention
    scores come out transposed (scoresT [k,q]) with no further transposes.
  - v computed in natural [t,c] layout (lhsT = xT chunk, rhs = wv), stored
    ones-augmented per head ([.., 65]) so the AV matmul also produces the
    softmax denominator in psum partition 64.
  - exp on ACT (scale folded in); no max-subtraction needed (logits ~N(0,1)).
  - normalization deferred: raw ctx parked in SBUF; one batched reciprocal
    over all 16 (head, q-window) denominator rows; partition-broadcast of the
    reciprocal rows via stride-0 DMA from a DRAM scratch; single multiply per
    window during the bf16 cast.
  - O-projection: lhsT = ctxT chunks, rhs = wo chunks, fp32 partial out.
All matmuls run in bf16 with fp32 psum accumulation.
"""
import sys

if "/opt/trn_rl_repo" not in sys.path:
    sys.path.insert(0, "/opt/trn_rl_repo")

import numpy as np

import concourse.bass as bass
import concourse.tile as tile
from concourse import bacc, mybir
from concourse.bass_utils import run_bass_kernel_spmd
from concourse.masks import make_identity

B, S, H = 4, 2048, 1024
NH, HD = 16, 64
G = H // 2            # local H columns per core
NHL = NH // 2         # local heads per core
P = 128
F32 = mybir.dt.float32
BF16 = mybir.dt.bfloat16
SCALE = 1.0 / float(np.sqrt(HD))

TT = S // P           # 16 token tiles
HC = H // P           # 8 contraction chunks for projections
CT = G // P           # 4 c-tiles
KC = S // P           # 16 k chunks
QW = 1024             # q window width in attention
NQH = S // QW         # 2 q windows
NW = NHL * NQH        # 16 (head, q-window) pairs
MM_N = 512            # matmul moving free dim (one psum bank)

_NC_CACHE = None


def _emit(nc, tc, aps):
    x, wq, wk, wv, wo, bq, bk, bv, out, rs_dram, rcp_dram = aps

    import contextlib
    ctx = contextlib.ExitStack()
    with ctx:
        persist = ctx.enter_context(tc.tile_pool(name="persist", bufs=1))

        # ---- persistent sbuf tensors ----
        qT = persist.tile([P, CT, S], BF16)
        kT = persist.tile([P, CT, S], BF16)
        v_aug = persist.tile([P, KC, NHL, HD + 1], BF16)
        ctxT = persist.tile([P, CT, S], BF16)
        wo_sb = persist.tile([P, CT, H], BF16)
        bq_sb = persist.tile([P, CT], F32)
        bk_sb = persist.tile([P, CT], F32)
        bv_row = persist.tile([1, G], BF16)
        ones_col = persist.tile([1, P], BF16)
        identity = persist.tile([P, P], F32)

        make_identity(nc, identity)
        nc.vector.memset(ones_col, 1.0)
        nc.vector.memset(v_aug[:, :, :, HD:HD + 1], 1.0)

        ph1 = tc.alloc_tile_pool(name="ph1", bufs=1)
        xT = ph1.tile([P, HC, S], BF16)
        wq_sb = ph1.tile([P, HC, G], BF16)
        wk_sb = ph1.tile([P, HC, G], BF16)
        wv_sb = ph1.tile([P, HC, G], BF16)

        # ---- phase 0: loads, x transpose, bf16 casts ----
        with tc.tile_pool(name="xload", bufs=3) as xload, \
             tc.tile_pool(name="wload", bufs=2) as wload, \
             tc.tile_pool(name="tp_ps", bufs=4, space="PSUM") as tp_ps:
            nc.sync.dma_start(out=bq_sb, in_=bq.rearrange("(ct p) -> p ct", p=P))
            nc.sync.dma_start(out=bk_sb, in_=bk.rearrange("(ct p) -> p ct", p=P))
            bv_f = wload.tile([1, G], F32, tag="bias")
            nc.sync.dma_start(out=bv_f, in_=bv.rearrange("(a c) -> a c", a=1))
            nc.vector.tensor_copy(out=bv_row, in_=bv_f)

            for tt in range(TT):
                xt = xload.tile([P, H], F32, tag="x")
                nc.sync.dma_start(out=xt, in_=x[tt * P:(tt + 1) * P, :])
                for hc in range(HC):
                    tp = tp_ps.tile([P, P], F32)
                    nc.tensor.transpose(tp, xt[:, hc * P:(hc + 1) * P], identity)
                    nc.vector.tensor_copy(
                        out=xT[:, hc, tt * P:(tt + 1) * P], in_=tp)

            for wap, dst in ((wq, wq_sb), (wk, wk_sb), (wv, wv_sb)):
                wt = wload.tile([P, HC, G], F32, tag="w")
                nc.sync.dma_start(
                    out=wt, in_=wap.rearrange("(hc p) c -> p hc c", p=P))
                nc.vector.tensor_copy(out=dst, in_=wt)
            wot = wload.tile([P, CT, H], F32, tag="w")
            nc.sync.dma_start(
                out=wot, in_=wo.rearrange("(cc p) o -> p cc o", p=P))
            nc.vector.tensor_copy(out=wo_sb, in_=wot)

        # ---- phases 1+2 interleaved: v first, then per c-tile qk proj and
        #      the two heads living in that c-tile ----
        with tc.tile_pool(name="proj_ps", bufs=2, space="PSUM") as pps, \
             tc.tile_pool(name="sc_ps", bufs=2, space="PSUM") as sc_pool, \
             tc.tile_pool(name="ctx_ps", bufs=1, space="PSUM") as ctx_pool, \
             tc.tile_pool(name="expp", bufs=3) as exp_pool:
            # v: [t, c] = sum_hc xT[hc, t]^T @ wv[hc, c]  (+ ones*bv)
            for tt in range(TT):
                acc = pps.tile([P, G], F32, tag="acc")
                for hc in range(HC):
                    nc.tensor.matmul(
                        acc,
                        lhsT=xT[:, hc, tt * P:(tt + 1) * P],
                        rhs=wv_sb[:, hc, :],
                        start=(hc == 0), stop=False)
                nc.tensor.matmul(
                    acc, lhsT=ones_col, rhs=bv_row, start=False, stop=True)
                nc.vector.tensor_copy(
                    out=v_aug[:, tt, :, 0:HD],
                    in_=acc.rearrange("p (h d) -> p h d", h=NHL))

            for ct in range(CT):
                # qT/kT for this c-tile
                for w_sb, b_sb, dst in ((wq_sb, bq_sb, qT), (wk_sb, bk_sb, kT)):
                    for nq in range(S // MM_N):
                        acc = pps.tile([P, MM_N], F32, tag="acc")
                        for hc in range(HC):
                            nc.tensor.matmul(
                                acc,
                                lhsT=w_sb[:, hc, ct * P:(ct + 1) * P],
                                rhs=xT[:, hc, nq * MM_N:(nq + 1) * MM_N],
                                start=(hc == 0), stop=(hc == HC - 1))
                        nc.vector.tensor_scalar_add(
                            out=dst[:, ct, nq * MM_N:(nq + 1) * MM_N],
                            in0=acc, scalar1=b_sb[:, ct:ct + 1])

                # attention for the two heads in this c-tile
                for h in (2 * ct, 2 * ct + 1):
                    po = (h % 2) * HD
                    for qh in range(NQH):
                        w_idx = h * NQH + qh
                        q0 = qh * QW
                        ctx_ps = ctx_pool.tile([HD + 1, QW], F32)
                        prev_ex = None
                        for kc in range(KC):
                            sc = sc_pool.tile([P, QW], F32)
                            for nq in range(QW // MM_N):
                                nc.tensor.matmul(
                                    sc[:, nq * MM_N:(nq + 1) * MM_N],
                                    lhsT=kT[po:po + HD, ct, kc * P:(kc + 1) * P],
                                    rhs=qT[po:po + HD, ct,
                                           q0 + nq * MM_N:q0 + (nq + 1) * MM_N],
                                    start=True, stop=True)
                            if prev_ex is not None:
                                pex, pkc = prev_ex
                                for nq in range(QW // MM_N):
                                    nc.tensor.matmul(
                                        ctx_ps[:, nq * MM_N:(nq + 1) * MM_N],
                                        lhsT=v_aug[:, pkc, h, :],
                                        rhs=pex[:, nq * MM_N:(nq + 1) * MM_N],
                                        start=(pkc == 0), stop=False)
                            ex = exp_pool.tile([P, QW], BF16)
                            nc.scalar.activation(
                                out=ex, in_=sc,
                                func=mybir.ActivationFunctionType.Exp,
                                scale=SCALE)
                            prev_ex = (ex, kc)
                        pex, pkc = prev_ex
                        for nq in range(QW // MM_N):
                            nc.tensor.matmul(
                                ctx_ps[:, nq * MM_N:(nq + 1) * MM_N],
                                lhsT=v_aug[:, pkc, h, :],
                                rhs=pex[:, nq * MM_N:(nq + 1) * MM_N],
                                start=False, stop=True)
                        # park raw ctx (bf16) + ship denom row to DRAM;
                        # normalization happens in a batched pass later
                        nc.vector.tensor_copy(
                            out=ctxT[po:po + HD, ct, q0:q0 + QW],
                            in_=ctx_ps[0:HD, :])
                        rs_stage = exp_pool.tile([1, QW], F32, tag="rs_stage")
                        nc.vector.tensor_copy(
                            out=rs_stage, in_=ctx_ps[HD:HD + 1, :])
                        nc.sync.dma_start(
                            out=rs_dram[w_idx:w_idx + 1, :], in_=rs_stage)

        ph1.release()

        # ---- phase 2.5: batched softmax normalization (in place on ctxT) ----
        with tc.tile_pool(name="normp", bufs=2) as norm_pool:
            rs_sb = norm_pool.tile([NW, QW], F32, tag="rs")
            nc.sync.dma_start(out=rs_sb, in_=rs_dram)
            rcp_sb = norm_pool.tile([NW, QW], F32, tag="rcp")
            nc.vector.reciprocal(out=rcp_sb, in_=rs_sb)
            rcp_bf = norm_pool.tile([NW, QW], BF16, tag="rcpb")
            nc.vector.tensor_copy(out=rcp_bf, in_=rcp_sb)
            nc.sync.dma_start(out=rcp_dram, in_=rcp_bf)
            for h in range(NHL):
                ct = h // 2
                po = (h % 2) * HD
                for qh in range(NQH):
                    w_idx = h * NQH + qh
                    row = rcp_dram[w_idx:w_idx + 1, :]
                    bcast = norm_pool.tile([P, QW], BF16, tag="bcast")
                    nc.sync.dma_start(
                        out=bcast[po:po + HD, :],
                        in_=bass.AP(tensor=row.tensor, offset=row.offset,
                                    ap=[[0, HD], [1, QW]]))
                    sl = ctxT[po:po + HD, ct, qh * QW:(qh + 1) * QW]
                    nc.vector.tensor_mul(out=sl, in0=sl, in1=bcast[po:po + HD, :])

        # ---- phase 3: output projection (partial; host sums group halves) ----
        with tc.tile_pool(name="o_ps", bufs=4, space="PSUM") as ops, \
             tc.tile_pool(name="o_sb", bufs=3) as osb:
            for tt in range(TT):
                for no in range(H // MM_N):
                    acc = ops.tile([P, MM_N], F32)
                    for cc in range(CT):
                        nc.tensor.matmul(
                            acc,
                            lhsT=ctxT[:, cc, tt * P:(tt + 1) * P],
                            rhs=wo_sb[:, cc, no * MM_N:(no + 1) * MM_N],
                            start=(cc == 0), stop=(cc == CT - 1))
                    ot = osb.tile([P, MM_N], F32)
                    nc.vector.tensor_copy(out=ot, in_=acc)
                    nc.sync.dma_start(
                        out=out[tt * P:(tt + 1) * P, no * MM_N:(no + 1) * MM_N],
                        in_=ot)


def build_program():
    global _NC_CACHE
    if _NC_CACHE is not None:
        return _NC_CACHE
    nc = bacc.Bacc("TRN2", debug=False, num_devices=8)
    x = nc.dram_tensor("x", [S, H], F32, kind="ExternalInput").ap()
    wq = nc.dram_tensor("wq", [H, G], F32, kind="ExternalInput").ap()
    wk = nc.dram_tensor("wk", [H, G], F32, kind="ExternalInput").ap()
    wv = nc.dram_tensor("wv", [H, G], F32, kind="ExternalInput").ap()
    wo = nc.dram_tensor("wo", [G, H], F32, kind="ExternalInput").ap()
    bq = nc.dram_tensor("bq", [G], F32, kind="ExternalInput").ap()
    bk = nc.dram_tensor("bk", [G], F32, kind="ExternalInput").ap()
    bv = nc.dram_tensor("bv", [G], F32, kind="ExternalInput").ap()
    out = nc.dram_tensor("out", [S, H], F32, kind="ExternalOutput").ap()
    rs_dram = nc.dram_tensor("rs_scratch", [NW, QW], F32).ap()
    rcp_dram = nc.dram_tensor("rcp_scratch", [NW, QW], BF16).ap()
    with tile.TileContext(nc) as tc:
        _emit(nc, tc, (x, wq, wk, wv, wo, bq, bk, bv, out, rs_dram, rcp_dram))
    nc.compile()
    _NC_CACHE = nc
    return nc


def make_in_maps(x, wq, bq, wk, bk, wv, bv, wo, bo):
    x = np.asarray(x, dtype=np.float32)
    in_maps = []
    for c in range(8):
        b, g = divmod(c, 2)
        sl = slice(g * G, (g + 1) * G)
        in_maps.append({
            "x": np.ascontiguousarray(x[b]),
            "wq": np.ascontiguousarray(np.asarray(wq, np.float32)[:, sl]),
            "wk": np.ascontiguousarray(np.asarray(wk, np.float32)[:, sl]),
            "wv": np.ascontiguousarray(np.asarray(wv, np.float32)[:, sl]),
            "wo": np.ascontiguousarray(np.asarray(wo, np.float32)[sl, :]),
            "bq": np.ascontiguousarray(np.asarray(bq, np.float32)[sl]),
            "bk": np.ascontiguousarray(np.asarray(bk, np.float32)[sl]),
            "bv": np.ascontiguousarray(np.asarray(bv, np.float32)[sl]),
        })
    return in_maps


def gather_out(results, bo):
    bo = np.asarray(bo, dtype=np.float32)
    out = np.empty((B, S, H), dtype=np.float32)
    for b in range(B):
        out[b] = results[2 * b]["out"] + results[2 * b + 1]["out"] + bo
    return out


def kernel(x, wq, bq, wk, bk, wv, bv, wo, bo, trace=False):
    nc = build_program()
    in_maps = make_in_maps(x, wq, bq, wk, bk, wv, bv, wo, bo)
    r = run_bass_kernel_spmd(nc, in_maps, list(range(8)), trace=trace)
    out = gather_out(r.results, bo)
    if trace:
        kernel.last_exec_time_ns = r.exec_time_ns
        kernel.last_results = r
    return out


# revision 19
# speedup vs baseline: 1.3178x; 1.0129x over previous
"""Multi-head attention (B=4, S=2048, H=1024, 16 heads) on 8 Trainium2 cores.

Sharding: core c = 2*b + g handles batch b with head-group g (8 heads = 512 of
1024 H-columns).  Each core computes Q/K/V projections for its column slice,
attention for its 8 heads, and a partial output projection against its 512
rows of wo.  The host sums the two partials per batch and adds bo.

Kernel internals (per core):
  - x [2048,1024] f32 is DMA'd in, PE-transposed to xT and cast to bf16.
  - qT/kT [c,t] computed directly (lhsT = wq chunk, rhs = xT) so attention
    scores come out transposed (scoresT [k,q]) with no further transposes.
  - v computed in natural [t,c] layout (lhsT = xT chunk, rhs = wv), stored
    ones-augmented per head ([.., 65]) so the AV matmul also produces the
    softmax denominator in psum partition 64.
  - exp on ACT (scale folded in); no max-subtraction needed (logits ~N(0,1)).
  - normalization deferred: raw ctx parked in SBUF; one batched reciprocal
    over all 16 (head, q-window) denominator rows; partition-broadcast of the
    reciprocal rows via stride-0 DMA from a DRAM scratch; single multiply per
    window during the bf16 cast.
  - O-projection: lhsT = ctxT chunks, rhs = wo chunks, fp32 partial out.
All matmuls run in bf16 with fp32 psum accumulation.
"""
import sys

if "/opt/trn_rl_repo" not in sys.path:
    sys.path.insert(0, "/opt/trn_rl_repo")

import numpy as np

import concourse.bass as bass
import concourse.tile as tile
from concourse import bacc, mybir
from concourse.bass_utils import run_bass_kernel_spmd
from concourse.masks import make_identity

B, S, H = 4, 2048, 1024
NH, HD = 16, 64
G = H // 2            # local H columns per core
NHL = NH // 2         # local heads per core
P = 128
F32 = mybir.dt.float32
BF16 = mybir.dt.bfloat16
SCALE = 1.0 / float(np.sqrt(HD))

TT = S // P           # 16 token tiles
HC = H // P           # 8 contraction chunks for projections
CT = G // P           # 4 c-tiles
KC = S // P           # 16 k chunks
QW = 1024             # q window width in attention
NQH = S // QW         # 2 q windows
NW = NHL * NQH        # 16 (head, q-window) pairs
MM_N = 512            # matmul moving free dim (one psum bank)

_NC_CACHE = None


def _emit(nc, tc, aps):
    x, wq, wk, wv, wo, bq, bk, bv, out, rs_dram, rcp_dram = aps

    import contextlib
    ctx = contextlib.ExitStack()
    with ctx:
        persist = ctx.enter_context(tc.tile_pool(name="persist", bufs=1))

        # ---- persistent sbuf tensors ----
        qT = persist.tile([P, CT, S], BF16)
        kT = persist.tile([P, CT, S], BF16)
        v_aug = persist.tile([P, KC, NHL, HD + 1], BF16)
        ctxT = persist.tile([P, CT, S], BF16)
        wo_sb = persist.tile([P, CT, H], BF16)
        bq_sb = persist.tile([P, CT], F32)
        bk_sb = persist.tile([P, CT], F32)
        bv_row = persist.tile([1, G], BF16)
        ones_col = persist.tile([1, P], BF16)
        identity = persist.tile([P, P], F32)

        make_identity(nc, identity)
        nc.vector.memset(ones_col, 1.0)
        nc.vector.memset(v_aug[:, :, :, HD:HD + 1], 1.0)

        ph1 = tc.alloc_tile_pool(name="ph1", bufs=1)
        xT = ph1.tile([P, HC, S], BF16)
        wq_sb = ph1.tile([P, HC, G], BF16)
        wk_sb = ph1.tile([P, HC, G], BF16)
        wv_sb = ph1.tile([P, HC, G], BF16)

        # psum pools for the whole kernel: proj/transpose accumulators share
        # slots (tag "acc"), scores double-buffered, ctx single
        pps = ctx.enter_context(tc.tile_pool(name="proj_ps", bufs=2, space="PSUM"))
        sc_pool = ctx.enter_context(tc.tile_pool(name="sc_ps", bufs=2, space="PSUM"))
        ctx_pool = ctx.enter_context(tc.tile_pool(name="ctx_ps", bufs=1, space="PSUM"))

        # ---- phase 0: loads, x transpose, bf16 casts ----
        with tc.tile_pool(name="xload", bufs=3) as xload, \
             tc.tile_pool(name="wload", bufs=2) as wload:
            nc.sync.dma_start(out=bq_sb, in_=bq.rearrange("(ct p) -> p ct", p=P))
            nc.sync.dma_start(out=bk_sb, in_=bk.rearrange("(ct p) -> p ct", p=P))
            bv_f = wload.tile([1, G], F32, tag="bias")
            nc.sync.dma_start(out=bv_f, in_=bv.rearrange("(a c) -> a c", a=1))
            nc.vector.tensor_copy(out=bv_row, in_=bv_f)

            for tt in range(TT):
                xt = xload.tile([P, H], F32, tag="x")
                nc.sync.dma_start(out=xt, in_=x[tt * P:(tt + 1) * P, :])
                for hc in range(HC):
                    tp = pps.tile([P, MM_N], F32, tag="acc")
                    nc.tensor.transpose(
                        tp[:, 0:P], xt[:, hc * P:(hc + 1) * P], identity)
                    nc.vector.tensor_copy(
                        out=xT[:, hc, tt * P:(tt + 1) * P], in_=tp[:, 0:P])

            for wap, dst in ((wq, wq_sb), (wk, wk_sb), (wv, wv_sb)):
                wt = wload.tile([P, HC, G], F32, tag="w")
                nc.sync.dma_start(
                    out=wt, in_=wap.rearrange("(hc p) c -> p hc c", p=P))
                nc.vector.tensor_copy(out=dst, in_=wt)
            wot = wload.tile([P, CT, H], F32, tag="w")
            nc.sync.dma_start(
                out=wot, in_=wo.rearrange("(cc p) o -> p cc o", p=P))
            nc.vector.tensor_copy(out=wo_sb, in_=wot)

        # ---- phases 1+2 interleaved: v first, then per c-tile qk proj and
        #      the two heads living in that c-tile ----
        with tc.tile_pool(name="expp", bufs=3) as exp_pool, \
             tc.tile_pool(name="normp", bufs=2) as norm_pool:
            # v: [t, c] = sum_hc xT[hc, t]^T @ wv[hc, c]  (+ ones*bv)
            for tt in range(TT):
                acc = pps.tile([P, G], F32, tag="acc")
                for hc in range(HC):
                    nc.tensor.matmul(
                        acc,
                        lhsT=xT[:, hc, tt * P:(tt + 1) * P],
                        rhs=wv_sb[:, hc, :],
                        start=(hc == 0), stop=False)
                nc.tensor.matmul(
                    acc, lhsT=ones_col, rhs=bv_row, start=False, stop=True)
                nc.vector.tensor_copy(
                    out=v_aug[:, tt, :, 0:HD],
                    in_=acc.rearrange("p (h d) -> p h d", h=NHL))

            for ct in range(CT):
                # qT/kT for this c-tile
                for w_sb, b_sb, dst in ((wq_sb, bq_sb, qT), (wk_sb, bk_sb, kT)):
                    for nq in range(S // MM_N):
                        acc = pps.tile([P, MM_N], F32, tag="acc")
                        for hc in range(HC):
                            nc.tensor.matmul(
                                acc,
                                lhsT=w_sb[:, hc, ct * P:(ct + 1) * P],
                                rhs=xT[:, hc, nq * MM_N:(nq + 1) * MM_N],
                                start=(hc == 0), stop=(hc == HC - 1))
                        nc.vector.tensor_scalar_add(
                            out=dst[:, ct, nq * MM_N:(nq + 1) * MM_N],
                            in0=acc, scalar1=b_sb[:, ct:ct + 1])

                # attention for the two heads in this c-tile
                for h in (2 * ct, 2 * ct + 1):
                    po = (h % 2) * HD
                    for qh in range(NQH):
                        w_idx = h * NQH + qh
                        q0 = qh * QW
                        ctx_ps = ctx_pool.tile([HD + 1, QW], F32)
                        prev_ex = None
                        for kc in range(KC):
                            sc = sc_pool.tile([P, QW], F32)
                            for nq in range(QW // MM_N):
                                nc.tensor.matmul(
                                    sc[:, nq * MM_N:(nq + 1) * MM_N],
                                    lhsT=kT[po:po + HD, ct, kc * P:(kc + 1) * P],
                                    rhs=qT[po:po + HD, ct,
                                           q0 + nq * MM_N:q0 + (nq + 1) * MM_N],
                                    start=True, stop=True)
                            if prev_ex is not None:
                                pex, pkc = prev_ex
                                for nq in range(QW // MM_N):
                                    nc.tensor.matmul(
                                        ctx_ps[:, nq * MM_N:(nq + 1) * MM_N],
                                        lhsT=v_aug[:, pkc, h, :],
                                        rhs=pex[:, nq * MM_N:(nq + 1) * MM_N],
                                        start=(pkc == 0), stop=False)
                            ex = exp_pool.tile([P, QW], BF16)
                            nc.scalar.activation(
                                out=ex, in_=sc,
                                func=mybir.ActivationFunctionType.Exp,
                                scale=SCALE)
                            prev_ex = (ex, kc)
                        pex, pkc = prev_ex
                        for nq in range(QW // MM_N):
                            nc.tensor.matmul(
                                ctx_ps[:, nq * MM_N:(nq + 1) * MM_N],
                                lhsT=v_aug[:, pkc, h, :],
                                rhs=pex[:, nq * MM_N:(nq + 1) * MM_N],
                                start=False, stop=True)
                        # park raw ctx (bf16) + ship denom row to DRAM;
                        # normalization happens in batched passes below
                        nc.vector.tensor_copy(
                            out=ctxT[po:po + HD, ct, q0:q0 + QW],
                            in_=ctx_ps[0:HD, :])
                        rs_stage = exp_pool.tile([1, QW], F32, tag="rs_stage")
                        nc.vector.tensor_copy(
                            out=rs_stage, in_=ctx_ps[HD:HD + 1, :])
                        nc.sync.dma_start(
                            out=rs_dram[w_idx:w_idx + 1, :], in_=rs_stage)

                # batched softmax normalization, half the windows at a time,
                # so the first batch overlaps the second half of attention
                if ct in (1, 3):
                    nb = ct // 2
                    wlo = nb * (NW // 2)
                    rs_sb = norm_pool.tile([NW // 2, QW], F32, tag="rs")
                    nc.sync.dma_start(
                        out=rs_sb, in_=rs_dram[wlo:wlo + NW // 2, :])
                    rcp_sb = norm_pool.tile([NW // 2, QW], F32, tag="rcp")
                    nc.vector.reciprocal(out=rcp_sb, in_=rs_sb)
                    rcp_bf = norm_pool.tile([NW // 2, QW], BF16, tag="rcpb")
                    nc.vector.tensor_copy(out=rcp_bf, in_=rcp_sb)
                    nc.sync.dma_start(
                        out=rcp_dram[wlo:wlo + NW // 2, :], in_=rcp_bf)
                    for h in range(4 * nb, 4 * nb + 4):
                        hct = h // 2
                        po = (h % 2) * HD
                        for qh in range(NQH):
                            w_idx = h * NQH + qh
                            row = rcp_dram[w_idx:w_idx + 1, :]
                            bcast = norm_pool.tile([P, QW], BF16, tag="bcast")
                            nc.sync.dma_start(
                                out=bcast[po:po + HD, :],
                                in_=bass.AP(tensor=row.tensor,
                                            offset=row.offset,
                                            ap=[[0, HD], [1, QW]]))
                            sl = ctxT[po:po + HD, hct, qh * QW:(qh + 1) * QW]
                            nc.vector.tensor_mul(
                                out=sl, in0=sl, in1=bcast[po:po + HD, :])

        ph1.release()

        # ---- phase 3: output projection (partial; host sums group halves) ----
        with tc.tile_pool(name="o_sb", bufs=3) as osb:
            for tt in range(TT):
                for no in range(H // MM_N):
                    acc = pps.tile([P, MM_N], F32, tag="acc")
                    for cc in range(CT):
                        nc.tensor.matmul(
                            acc,
                            lhsT=ctxT[:, cc, tt * P:(tt + 1) * P],
                            rhs=wo_sb[:, cc, no * MM_N:(no + 1) * MM_N],
                            start=(cc == 0), stop=(cc == CT - 1))
                    ot = osb.tile([P, MM_N], F32)
                    nc.vector.tensor_copy(out=ot, in_=acc)
                    nc.sync.dma_start(
                        out=out[tt * P:(tt + 1) * P, no * MM_N:(no + 1) * MM_N],
                        in_=ot)


def build_program():
    global _NC_CACHE
    if _NC_CACHE is not None:
        return _NC_CACHE
    nc = bacc.Bacc("TRN2", debug=False, num_devices=8)
    x = nc.dram_tensor("x", [S, H], F32, kind="ExternalInput").ap()
    wq = nc.dram_tensor("wq", [H, G], F32, kind="ExternalInput").ap()
    wk = nc.dram_tensor("wk", [H, G], F32, kind="ExternalInput").ap()
    wv = nc.dram_tensor("wv", [H, G], F32, kind="ExternalInput").ap()
    wo = nc.dram_tensor("wo", [G, H], F32, kind="ExternalInput").ap()
    bq = nc.dram_tensor("bq", [G], F32, kind="ExternalInput").ap()
    bk = nc.dram_tensor("bk", [G], F32, kind="ExternalInput").ap()
    bv = nc.dram_tensor("bv", [G], F32, kind="ExternalInput").ap()
    out = nc.dram_tensor("out", [S, H], F32, kind="ExternalOutput").ap()
    rs_dram = nc.dram_tensor("rs_scratch", [NW, QW], F32).ap()
    rcp_dram = nc.dram_tensor("rcp_scratch", [NW, QW], BF16).ap()
    with tile.TileContext(nc) as tc:
        _emit(nc, tc, (x, wq, wk, wv, wo, bq, bk, bv, out, rs_dram, rcp_dram))
    nc.compile()
    _NC_CACHE = nc
    return nc


def make_in_maps(x, wq, bq, wk, bk, wv, bv, wo, bo):
    x = np.asarray(x, dtype=np.float32)
    in_maps = []
    for c in range(8):
        b, g = divmod(c, 2)
        sl = slice(g * G, (g + 1) * G)
        in_maps.append({
            "x": np.ascontiguousarray(x[b]),
            "wq": np.ascontiguousarray(np.asarray(wq, np.float32)[:, sl]),
            "wk": np.ascontiguousarray(np.asarray(wk, np.float32)[:, sl]),
            "wv": np.ascontiguousarray(np.asarray(wv, np.float32)[:, sl]),
            "wo": np.ascontiguousarray(np.asarray(wo, np.float32)[sl, :]),
            "bq": np.ascontiguousarray(np.asarray(bq, np.float32)[sl]),
            "bk": np.ascontiguousarray(np.asarray(bk, np.float32)[sl]),
            "bv": np.ascontiguousarray(np.asarray(bv, np.float32)[sl]),
        })
    return in_maps


def gather_out(results, bo):
    bo = np.asarray(bo, dtype=np.float32)
    out = np.empty((B, S, H), dtype=np.float32)
    for b in range(B):
        out[b] = results[2 * b]["out"] + results[2 * b + 1]["out"] + bo
    return out


def kernel(x, wq, bq, wk, bk, wv, bv, wo, bo, trace=False):
    nc = build_program()
    in_maps = make_in_maps(x, wq, bq, wk, bk, wv, bv, wo, bo)
    r = run_bass_kernel_spmd(nc, in_maps, list(range(8)), trace=trace)
    out = gather_out(r.results, bo)
    if trace:
        kernel.last_exec_time_ns = r.exec_time_ns
        kernel.last_results = r
    return out
